# revision 11
# baseline (speedup 1.0000x reference)
"""Masked multi-head self-attention kernel for 8 Trainium2 NeuronCores.

Full module: qkv projection -> causal softmax attention (16 heads) -> out
projection, for x[4, 2048, 1024].

Sharding: core c handles batch b = c//2 and heads h0 = (c%2)*8 .. h0+8.
QKV projection + attention are fully local to a core.  The out projection
contracts over all 16 heads' channels, so the two cores of a batch exchange
their attention outputs with pairwise AllGathers (grouped by heads
{0-3},{4,5},{6},{7} per query block for overlap) and each computes half of
the output columns.  Each core returns out[b][:, half].T ([512, 2048]); the
host reassembles.

Schedule notes (what makes this fast):
 - The attention inner loop is ACT(exp)-bound.  Score k-tiles are paired
   into 2-bank [128,1024] PSUM tiles so one exp activation covers two
   tiles, amortizing the per-instruction overhead.
 - Scores/PV matmuls are issue-interleaved per head-slot (pv of head h-1
   between score pairs of head h) so PSUM WAR waits do not idle the PE.
 - Diagonal tiles get restricted exp/PV column ranges; causal masks are
   [128,128] affine_selects on gpsimd (off the critical path: PV of a
   head runs one slot after its scores).
 - Everything is fp16 (same PE speed as bf16, 8x the mantissa).
 - Stage 1 runs tc-outer so the first matmul only waits for 1/4 of x.
 - The last query block's out-projection partially accumulates per gather
   group so only the final {head 7} gather + 2 k-chunks are exposed.
"""

import math
import os
import sys

for _p in ("/opt/trn_rl_repo", "/root/.axon_site/_ro/trn_rl_repo"):
    if os.path.isdir(_p) and _p not in sys.path:
        sys.path.insert(0, _p)
        break

import ml_dtypes
import numpy as np

import concourse.bass as bass
import concourse.mybir as mybir
import concourse.tile as tile
from concourse import bacc
from concourse.bass_utils import run_bass_kernel_spmd

B, T, C, H = 4, 2048, 1024, 16
D = 64                 # head dim
NCORES = 8
HPC = H // 2           # heads per core = 8
CPC = HPC * D          # channels per core = 512
P = 128                # partitions
QB = 512               # query block
NQB = T // QB          # 4
KC = C // P            # contraction chunks for C = 8
SCALE = 1.0 / math.sqrt(D)

F32 = mybir.dt.float32
F16 = mybir.dt.float16
EXP = mybir.ActivationFunctionType.Exp

_CACHE = {}

# gather groups: heads 0-3, 4-5, 6, 7
GGRP = [(0, 4), (4, 6), (6, 7), (7, 8)]
GRP_OF = {}
for _gi, (_s, _e) in enumerate(GGRP):
    for _h in range(_s, _e):
        GRP_OF[_h] = (_gi, _h - _s)
# out-proj contraction chunk cc -> (gather buffer, sub-chunk)
CCMAP = [(0, 0), (0, 1), (0, 2), (0, 3), (1, 0), (1, 1), (2, 0), (3, 0)]


def build():
    nc = bacc.Bacc("TRN2", num_devices=NCORES, debug=False)

    xT = nc.dram_tensor("xT", [C, T], F16, kind="ExternalInput")
    wqk = nc.dram_tensor("wqk", [C, 2 * CPC], F16, kind="ExternalInput")
    wv = nc.dram_tensor("wv", [C, CPC], F16, kind="ExternalInput")
    bqk = nc.dram_tensor("bqk", [1, 2 * CPC], F32, kind="ExternalInput")
    wout = nc.dram_tensor("wout", [C, CPC], F16, kind="ExternalInput")
    bout = nc.dram_tensor("bout", [1, CPC], F32, kind="ExternalInput")
    outT = nc.dram_tensor("outT", [CPC, T], F32, kind="ExternalOutput")

    groups = [[0, 1], [2, 3], [4, 5], [6, 7]]

    with tile.TileContext(nc) as tc:
        with (
            tc.tile_pool(name="const", bufs=1) as constp,
            tc.tile_pool(name="ytp", bufs=1) as ytp,
            tc.tile_pool(name="vaugp", bufs=1) as vaugp,
            tc.tile_pool(name="dram", bufs=1, space="DRAM") as dramp,
        ):
            # per-partition bias layouts: bq_sb[p, n] = bqk[n*128 + p]
            bq_sb = constp.tile([P, 8], F32, tag="bq")
            nc.sync.dma_start(
                bq_sb[:].rearrange("p (o n) -> p o n", o=1),
                bqk.ap().rearrange("o (n p) -> p o n", p=P),
            )
            bo_sb = constp.tile([P, 4], F32, tag="bo")
            nc.sync.dma_start(
                bo_sb[:].rearrange("p (o n) -> p o n", o=1),
                bout.ap().rearrange("o (n p) -> p o n", p=P),
            )
            ones_f32 = constp.tile([P, P], F32, tag="ones")
            nc.vector.memset(ones_f32[:], 1.0)

            # Q^T,K^T: 8 chunks of [128 ch, 2048 t] (Q: 0-3, K: 4-7)
            yts = [
                ytp.tile([P, T], F16, name=f"yt{n}", tag=f"yt{n}")
                for n in range(8)
            ]
            # V natural (+ones col) per head h at [h, tt, 0:65]
            vaug_all = vaugp.tile([P, HPC * 16 * 65], F16, tag="vaug")
            vaug4 = vaug_all[:].rearrange("p (h k c) -> p h k c", h=HPC, c=65)
            # ones column at c=64 for every (h, tt): one strided copy
            nc.vector.tensor_copy(
                vaug_all[:]
                .rearrange("p (k c) -> p k c", c=65)[:, :, 64:65],
                ones_f32[:, 0:HPC * 16].rearrange("p (a b) -> p a b", b=1),
            )

            # ---------------- stage 1: qkv projection, V ----------------
            with (
                tc.tile_pool(name="xtp", bufs=1) as xtp,
                tc.tile_pool(name="wqp", bufs=1) as wqp,
                tc.tile_pool(name="wvp", bufs=1) as wvp,
                tc.tile_pool(name="ps_y", bufs=4, space="PSUM") as psy,
                tc.tile_pool(name="ps_v", bufs=2, space="PSUM") as psv,
            ):
                # x^T in [128, 512] slices, t-major load order
                xts = [
                    [
                        xtp.tile([P, QB], F16, name=f"xt{kc}_{t4}",
                                 tag=f"xt{kc}_{t4}")
                        for t4 in range(4)
                    ]
                    for kc in range(KC)
                ]
                wq_sb = [
                    wqp.tile([P, 2 * CPC], F16, name=f"wq{kc}", tag=f"wq{kc}")
                    for kc in range(KC)
                ]
                wv_sb = [
                    wvp.tile([P, CPC], F16, name=f"wv{kc}", tag=f"wv{kc}")
                    for kc in range(KC)
                ]
                # DMA order: x slices for t4=0, qk weights, rest of x, v wts
                for kc in range(KC):
                    nc.sync.dma_start(
                        xts[kc][0][:], xT[kc * P:(kc + 1) * P, 0:QB]
                    )
                for kc in range(KC):
                    nc.sync.dma_start(
                        wq_sb[kc][:], wqk[kc * P:(kc + 1) * P, :]
                    )
                for t4 in range(1, 4):
                    for kc in range(KC):
                        nc.sync.dma_start(
                            xts[kc][t4][:],
                            xT[kc * P:(kc + 1) * P, t4 * QB:(t4 + 1) * QB],
                        )
                for kc in range(KC):
                    nc.sync.dma_start(
                        wv_sb[kc][:], wv[kc * P:(kc + 1) * P, :]
                    )

                for t4 in range(4):
                    for n in range(8):
                        py = psy.tile([P, QB], F32, tag="py")
                        for kc in range(KC):
                            nc.tensor.matmul(
                                py[:],
                                wq_sb[kc][:, n * P:(n + 1) * P],
                                xts[kc][t4][:],
                                start=(kc == 0),
                                stop=(kc == KC - 1),
                            )
                        nc.vector.tensor_scalar_add(
                            yts[n][:, t4 * QB:(t4 + 1) * QB],
                            py[:],
                            bq_sb[:, n:n + 1],
                        )
                    for j in range(4):
                        tt = t4 * 4 + j
                        pv = psv.tile([P, CPC], F32, tag="pv")
                        for kc in range(KC):
                            nc.tensor.matmul(
                                pv[:],
                                xts[kc][t4][:, j * P:(j + 1) * P],
                                wv_sb[kc][:],
                                start=(kc == 0),
                                stop=(kc == KC - 1),
                            )
                        # scatter pv's 8 head-chunks into vaug_all, one instr
                        nc.vector.tensor_copy(
                            vaug4[:, :, tt, 0:64],
                            pv[:].rearrange("p (h c) -> p h c", c=64),
                        )

            # ---------------- stage 2+3: attention, gather, out proj ----
            with (
                tc.tile_pool(name="ptp", bufs=3) as ptp,
                tc.tile_pool(name="recip", bufs=4) as recipp,
                tc.tile_pool(name="bc", bufs=3) as bcp,
                tc.tile_pool(name="atv", bufs=3) as atvp,
                tc.tile_pool(name="w2", bufs=1) as w2p,
                tc.tile_pool(name="agr", bufs=4) as agrp,
                tc.tile_pool(name="outsb", bufs=4) as outsbp,
                tc.tile_pool(name="ps_s", bufs=2, space="PSUM") as pss,
                tc.tile_pool(name="ps_a", bufs=2, space="PSUM") as psa,
                tc.tile_pool(name="ps_o", bufs=2, space="PSUM") as pso,
            ):
                w2sb = w2p.tile([P, KC * CPC], F16, tag="w2")
                nc.sync.dma_start(
                    w2sb[:].rearrange("p (c n) -> p c n", n=CPC),
                    wout.ap().rearrange("(c p) n -> p c n", p=P),
                )
                w23 = w2sb[:].rearrange("p (c n) -> p c n", n=CPC)

                def s_pairs(qb, h, ptreg):
                    """Yield score-pair steps for (qb, h); returns pt slice
                    info via pts dict {kt: (col, qoff)}.
                    Region layout: diag pairs first, then off-diag pairs."""
                    qt = yts[h // 2]
                    kt_c = yts[4 + h // 2]
                    poff = (h % 2) * 64
                    diags = [(4 * qb + j, j * P) for j in range(4)]
                    offs = [(kt, 0) for kt in range(4 * qb)]
                    tiles = diags + offs
                    pairs = [
                        (tiles[i], tiles[i + 1])
                        for i in range(0, len(tiles), 2)
                    ]
                    pts = {}
                    for pi, ((kta, qa), (ktb, qb_)) in enumerate(pairs):
                        col = pi * 2 * QB
                        pts[kta] = (col, qa)
                        pts[ktb] = (col + QB, qb_)

                    def step(pi):
                        (kta, qa), (ktb, qb_) = pairs[pi]
                        col = pi * 2 * QB
                        sc = pss.tile([P, 2 * QB], F32, tag="sc")
                        for half, (kt, qo) in enumerate(
                            ((kta, qa), (ktb, qb_))
                        ):
                            nc.tensor.matmul(
                                sc[:, half * QB + qo:(half + 1) * QB],
                                kt_c[poff:poff + 64, kt * P:(kt + 1) * P],
                                qt[poff:poff + 64,
                                   qb * QB + qo:(qb + 1) * QB],
                                start=True, stop=True,
                            )
                        if qa == 0 and qb_ == 0:
                            # full-width pair: one wide exp over both banks
                            nc.scalar.activation(
                                ptreg[:, col:col + 2 * QB],
                                sc[:],
                                EXP, scale=SCALE,
                            )
                        else:
                            for half, qo in enumerate((qa, qb_)):
                                nc.scalar.activation(
                                    ptreg[:, half * QB + col + qo:
                                          half * QB + col + QB],
                                    sc[:, half * QB + qo:(half + 1) * QB],
                                    EXP, scale=SCALE,
                                )
                        # causal masks for diagonal tiles in this pair
                        for half, (kt, qo) in enumerate(
                            ((kta, qa), (ktb, qb_))
                        ):
                            j = kt - 4 * qb
                            if j >= 0:
                                blk = col + half * QB + j * P
                                nc.gpsimd.affine_select(
                                    out=ptreg[:, blk:blk + P],
                                    in_=ptreg[:, blk:blk + P],
                                    compare_op=mybir.AluOpType.is_ge,
                                    fill=0.0,
                                    base=0,
                                    pattern=[[1, P]],
                                    channel_multiplier=-1,
                                )

                    return [lambda pi=pi: step(pi) for pi in range(len(pairs))], pts

                def pv_chunks(qb, h, ptreg, pts):
                    """PV matmul thunks for head h (2 mms per chunk), plus a
                    finish thunk (norm + atv DMA).  Order: diag j0 (full,
                    start) -> off-diags -> diag j1..j3 (restricted)."""
                    order = (
                        [4 * qb]
                        + list(range(0, 4 * qb))
                        + [4 * qb + j for j in (1, 2, 3)]
                    )
                    pa = psa.tile([P, QB], F32, tag="pa")

                    def mk(i):
                        def mm():
                            kt = order[i]
                            col, qo = pts[kt]
                            # ragged accumulation ranges: start zeroes only
                            # written elements (hw semantics); group check
                            # would reject the subrange stop
                            nc.tensor.matmul(
                                pa[0:65, qo:QB],
                                vaug4[:, h, kt, :],
                                ptreg[:, col + qo:col + QB],
                                start=(i == 0),
                                stop=(i == len(order) - 1),
                                skip_group_check=True,
                            )
                        return mm

                    return [mk(i) for i in range(len(order))], pa

                def norm_and_send(qb, h, pa, ag_in):
                    gi, row = GRP_OF[h]
                    # reciprocal_approx_fast misreads PSUM at partition
                    # offset 64 — stage through SBUF at partition 0
                    sums = recipp.tile([1, QB], F32, tag="sums")
                    nc.vector.tensor_copy(sums[:], pa[64:65, :])
                    recip = recipp.tile([1, QB], F32, tag="recip")
                    nc.vector.reciprocal_approx_fast(recip[:], sums[:])
                    bc = bcp.tile([64, QB], F32, tag="bc")
                    nc.gpsimd.partition_broadcast(bc[:], recip[:])
                    atv = atvp.tile([64, QB], F16, tag="atv")
                    nc.vector.tensor_mul(atv[:], pa[0:64, :], bc[:])
                    nc.sync.dma_start(
                        ag_in[row * 64:(row + 1) * 64, :], atv[:]
                    )

                def gather(ag_in, ag_out):
                    nc.gpsimd.collective_compute(
                        "AllGather",
                        mybir.AluOpType.bypass,
                        replica_groups=groups,
                        ins=[ag_in.opt()],
                        outs=[ag_out.opt()],
                    )

                def load_agr(qb, gi, ag_out):
                    ncch = 2 * (GGRP[gi][1] - GGRP[gi][0]) * 64 // P
                    agr = agrp.tile(
                        [P, ncch * QB], F16,
                        name=f"agr{qb}_{gi}", tag=f"agr{gi}",
                    )
                    nc.sync.dma_start(
                        agr[:].rearrange("p (c n) -> p c n", n=QB),
                        ag_out[:].rearrange("(c p) n -> p c n", p=P),
                    )
                    return agr[:].rearrange("p (c n) -> p c n", n=QB)

                def outproj_oc(qb, oc, agr3s, ccs, po, start, stop):
                    for idx, cc in enumerate(ccs):
                        gi, sub = CCMAP[cc]
                        nc.tensor.matmul(
                            po[:],
                            w23[:, cc, oc * P:(oc + 1) * P],
                            agr3s[gi][:, sub, :],
                            start=(start and idx == 0),
                            stop=(stop and idx == len(ccs) - 1),
                        )

                def outproj_finish(qb, oc, po):
                    osb = outsbp.tile([P, QB], F32, tag="osb")
                    nc.vector.tensor_scalar_add(
                        osb[:], po[:], bo_sb[:, oc:oc + 1]
                    )
                    nc.sync.dma_start(
                        outT[oc * P:(oc + 1) * P, qb * QB:(qb + 1) * QB],
                        osb[:],
                    )

                def outproj_full(qb, agr3s):
                    for oc in range(4):
                        po = pso.tile([P, QB], F32, tag="po")
                        outproj_oc(qb, oc, agr3s, list(range(KC)), po,
                                   True, True)
                        outproj_finish(qb, oc, po)

                # ---------------- main loop ----------------
                ag_ins_q, ag_outs_q = {}, {}
                for qb in range(NQB):
                    ag_ins_q[qb] = [
                        dramp.tile(
                            [(e - s) * 64, QB], F16,
                            name=f"agin{qb}_{i}", tag=f"agin{qb}_{i}",
                        )
                        for i, (s, e) in enumerate(GGRP)
                    ]
                    ag_outs_q[qb] = [
                        dramp.tile(
                            [2 * (e - s) * 64, QB], F16,
                            name=f"agout{qb}_{i}", tag=f"agout{qb}_{i}",
                        )
                        for i, (s, e) in enumerate(GGRP)
                    ]

                slots = [(qb, h) for qb in range(NQB) for h in range(HPC)]
                prev = None            # (qb, h, ptreg, pts)
                qb3_state = {}
                for qb, h in slots:
                    # uniform max size so the pool slots don't grow
                    ptreg = ptp.tile(
                        [P, 4 * NQB * QB], F16,
                        name=f"pt{qb}_{h}", tag="ptreg",
                    )
                    steps, pts = s_pairs(qb, h, ptreg)
                    pvs, pa = [], None
                    if prev is not None:
                        pqb, ph, pregion, ppts = prev
                        pvs, pa = pv_chunks(pqb, ph, pregion, ppts)
                    # interleave: score-pair step, then 2 pv mms
                    pi = vi = 0
                    while pi < len(steps) or vi < len(pvs):
                        if pi < len(steps):
                            steps[pi]()
                            pi += 1
                        for _ in range(2):
                            if vi < len(pvs):
                                pvs[vi]()
                                vi += 1
                    if prev is not None:
                        pqb, ph, _, _ = prev
                        gi = GRP_OF[ph][0]
                        norm_and_send(pqb, ph, pa, ag_ins_q[pqb][gi])
                        if ph == GGRP[gi][1] - 1:
                            gather(ag_ins_q[pqb][gi], ag_outs_q[pqb][gi])
                    prev = (qb, h, ptreg, pts)

                    # deferred whole out-projection for qb-1 (all its
                    # gathers fired by slot (qb, 1))
                    if h == 2 and qb in (1, 2, 3):
                        agr3s = [
                            load_agr(qb - 1, gi, ag_outs_q[qb - 1][gi])
                            for gi in range(4)
                        ]
                        outproj_full(qb - 1, agr3s)
                    if qb == 3:
                        if h == 5:
                            # g0 of qb3 gathered (fired at slot (3,4)):
                            # start partial accumulation for oc 0,1
                            agr3s = qb3_state.setdefault("agr3s", {})
                            agr3s[0] = load_agr(3, 0, ag_outs_q[3][0])
                            po01 = []
                            for oc in (0, 1):
                                po = pso.tile([P, QB], F32, tag="po")
                                outproj_oc(3, oc, agr3s, [0, 1, 2, 3],
                                           po, True, False)
                                po01.append(po)
                            qb3_state["po01"] = po01
                        if h == 7:
                            agr3s = qb3_state["agr3s"]
                            agr3s[1] = load_agr(3, 1, ag_outs_q[3][1])
                            for oc in (0, 1):
                                outproj_oc(3, oc, agr3s, [4, 5],
                                           qb3_state["po01"][oc],
                                           False, False)

                # close out: pv + norm + gather for the last head
                pqb, ph, pregion, ppts = prev
                pvs, pa = pv_chunks(pqb, ph, pregion, ppts)
                for mm in pvs:
                    mm()
                norm_and_send(pqb, ph, pa, ag_ins_q[pqb][3])
                gather(ag_ins_q[pqb][3], ag_outs_q[pqb][3])

                # tail: finish oc 0,1 with cc 6,7; oc 2,3 full
                agr3s = qb3_state["agr3s"]
                agr3s[2] = load_agr(3, 2, ag_outs_q[3][2])
                agr3s[3] = load_agr(3, 3, ag_outs_q[3][3])
                for oc in (0, 1):
                    outproj_oc(3, oc, agr3s, [6, 7],
                               qb3_state["po01"][oc], False, True)
                    outproj_finish(3, oc, qb3_state["po01"][oc])
                for oc in (2, 3):
                    po = pso.tile([P, QB], F32, tag="po")
                    outproj_oc(3, oc, agr3s, list(range(KC)), po,
                               True, True)
                    outproj_finish(3, oc, po)

    nc.compile()
    return nc


def kernel(x, w_qkv, b_qkv, w_out, b_out):
    x = np.asarray(x, dtype=np.float32)
    w_qkv = np.asarray(w_qkv, dtype=np.float32)
    b_qkv = np.asarray(b_qkv, dtype=np.float32)
    w_out = np.asarray(w_out, dtype=np.float32)
    b_out = np.asarray(b_out, dtype=np.float32)

    if "nc" not in _CACHE:
        _CACHE["nc"] = build()
    nc = _CACHE["nc"]

    # V bias passes through softmax unchanged; fold it into the out bias
    bv_all = b_qkv[2 * C:3 * C]

    in_maps = []
    for c in range(NCORES):
        b = c // 2
        h0 = (c % 2) * HPC
        cols = slice(h0 * D, h0 * D + CPC)
        wqk_c = np.concatenate(
            [w_qkv[:, cols], w_qkv[:, C:][:, cols]], axis=1
        )
        wv_c = w_qkv[:, 2 * C:][:, cols]
        bqk_c = np.concatenate(
            [b_qkv[cols], b_qkv[C:][cols]]
        ).reshape(1, 2 * CPC)
        half = slice((c % 2) * CPC, (c % 2) * CPC + CPC)
        wo = w_out[:, half]
        # rows permuted to the gathered channel order:
        # [even h0-3, odd h0-3, even h4-5, odd h4-5, e h6, o h6, e h7, o h7]
        wo_perm = np.concatenate(
            [wo[0:256], wo[512:768],
             wo[256:384], wo[768:896],
             wo[384:448], wo[896:960],
             wo[448:512], wo[960:1024]], axis=0
        )
        bout_eff = b_out[half] + bv_all @ w_out[:, half]
        in_maps.append({
            "xT": np.ascontiguousarray(x[b].T.astype(np.float16)),
            "wqk": np.ascontiguousarray(wqk_c.astype(np.float16)),
            "wv": np.ascontiguousarray(wv_c.astype(np.float16)),
            "bqk": np.ascontiguousarray(bqk_c),
            "wout": np.ascontiguousarray(wo_perm.astype(np.float16)),
            "bout": np.ascontiguousarray(bout_eff).reshape(1, CPC),
        })

    kwargs = {}
    tdir = os.environ.get("KERNEL_TRACE_DIR")
    if tdir:
        kwargs = dict(trace=True, tmpdir=tdir)
    res = run_bass_kernel_spmd(
        nc, in_maps, core_ids=list(range(NCORES)), **kwargs
    )
    _CACHE["last_results"] = res

    out = np.empty((B, T, C), dtype=np.float32)
    for c in range(NCORES):
        b = c // 2
        half = slice((c % 2) * CPC, (c % 2) * CPC + CPC)
        out[b][:, half] = res.results[c]["outT"].T
    return out
